# revision 3
# baseline (speedup 1.0000x reference)
"""Trainium2 Bass kernel for nn_AxonalConnections.

Computes, for full inputs v1, v2 of shape [32, 1024, 1024] and four
[512, 512] weight maps:
    hub = v1[:, ::2, ::2] * w_v1_hub + v2[:, ::2, ::2] * w_v2_hub
    out = v1[:, ::2, ::2] * w_v1_out + v2[:, ::2, ::2] * w_v2_out

Sharding: data-parallel over the batch dim across 8 NeuronCores
(4 images per core); weight maps replicated on every core.

Per-core kernel (memory-bound design):
  - Only even source rows are read from HBM (stride-2 row DMA, 4 KiB
    contiguous chunks): 512 rows -> 4 row-blocks of 128 partitions.
  - The even-column gather is folded into the DVE multiply as a
    stride-2 free-dim access pattern (no separate gather pass).
  - Tiles pack all 4 images along the free dim so each DVE op runs at
    FD=2048 (amortizes the per-op overhead) and each DMA is >= 1 MiB.
"""

import sys

if "/opt/trn_rl_repo" not in sys.path:
    sys.path.insert(0, "/opt/trn_rl_repo")

import numpy as np

N_CORES = 8
B_FULL = 32
B_CORE = B_FULL // N_CORES  # 4 images per core
SH = SW = 1024
TH = TW = 512
RB = 4  # row blocks of 128 partitions each (512 target rows / 128)

_W_NAMES = ("w_v1_hub", "w_v2_hub", "w_v1_out", "w_v2_out")

_nc_cache = {}


def build_nc(b=B_CORE, sh=SH, sw=SW, th=TH, tw=TW, rb_count=RB):
    """Build the per-core Bass program. Parameterized so a miniature
    version can be validated in CoreSim."""
    from concourse import bacc, mybir
    from concourse.tile import TileContext

    f32 = mybir.dt.float32
    nc = bacc.Bacc("TRN2", target_bir_lowering=False, debug=False,
                   num_devices=N_CORES)

    v1 = nc.declare_dram_parameter("v1", [b, sh, sw], f32, isOutput=False)
    v2 = nc.declare_dram_parameter("v2", [b, sh, sw], f32, isOutput=False)
    ws = {
        name: nc.declare_dram_parameter(name, [th, tw], f32, isOutput=False)
        for name in _W_NAMES
    }
    hub = nc.declare_dram_parameter("hub", [b, th, tw], f32, isOutput=True)
    out = nc.declare_dram_parameter("out", [b, th, tw], f32, isOutput=True)

    p = th // rb_count  # partitions per row block (128 at full size)

    with TileContext(nc) as tc:
        with tc.tile_pool(name="wpool", bufs=1) as wpool, \
             tc.tile_pool(name="inpool", bufs=2) as inpool, \
             tc.tile_pool(name="opool", bufs=2) as opool, \
             tc.tile_pool(name="tpool", bufs=2) as tpool:
            # Resident weights: [p, rb, tw] (partition-major row blocks).
            wt = {}
            for name in _W_NAMES:
                t = wpool.tile([p, rb_count, tw], f32, tag=name)
                nc.sync.dma_start(
                    out=t, in_=ws[name].rearrange("(a q) c -> q a c", q=p))
                wt[name] = t

            for rb in range(rb_count):
                r0 = 2 * p * rb  # first source row of this block
                tv1 = inpool.tile([p, b, sw], f32, tag="tv1")
                tv2 = inpool.tile([p, b, sw], f32, tag="tv2")
                nc.sync.dma_start(
                    out=tv1,
                    in_=v1[:, r0:r0 + 2 * p:2, :].transpose([1, 0, 2]))
                nc.sync.dma_start(
                    out=tv2,
                    in_=v2[:, r0:r0 + 2 * p:2, :].transpose([1, 0, 2]))
                v1e = tv1[:, :, 0:sw:2]  # [p, b, tw] stride-2 col gather
                v2e = tv2[:, :, 0:sw:2]

                for dram_dst, w1n, w2n, otag in (
                        (hub, "w_v1_hub", "w_v2_hub", "thub"),
                        (out, "w_v1_out", "w_v2_out", "tout")):
                    to = opool.tile([p, b, tw], f32, tag=otag)
                    tt = tpool.tile([p, b, tw], f32, tag="tmp")
                    w1 = wt[w1n][:, rb:rb + 1, :].broadcast_to([p, b, tw])
                    w2 = wt[w2n][:, rb:rb + 1, :].broadcast_to([p, b, tw])
                    nc.vector.tensor_mul(out=to, in0=v1e, in1=w1)
                    nc.vector.tensor_mul(out=tt, in0=v2e, in1=w2)
                    nc.vector.tensor_add(out=to, in0=to, in1=tt)
                    nc.sync.dma_start(
                        out=dram_dst[:, p * rb:p * rb + p, :].transpose(
                            [1, 0, 2]),
                        in_=to)

    nc.compile()
    return nc


def _get_nc():
    if "full" not in _nc_cache:
        _nc_cache["full"] = build_nc()
    return _nc_cache["full"]


def kernel(v1, v2, w_v1_hub, w_v2_hub, w_v1_out, w_v2_out, **run_kwargs):
    """Full-input entry point: shards over batch, runs on 8 cores,
    gathers full outputs. Returns (hub, out) like the reference."""
    from concourse.bass_utils import run_bass_kernel_spmd

    nc = _get_nc()
    v1 = np.ascontiguousarray(np.asarray(v1, dtype=np.float32))
    v2 = np.ascontiguousarray(np.asarray(v2, dtype=np.float32))
    weights = {
        "w_v1_hub": np.ascontiguousarray(np.asarray(w_v1_hub, np.float32)),
        "w_v2_hub": np.ascontiguousarray(np.asarray(w_v2_hub, np.float32)),
        "w_v1_out": np.ascontiguousarray(np.asarray(w_v1_out, np.float32)),
        "w_v2_out": np.ascontiguousarray(np.asarray(w_v2_out, np.float32)),
    }

    core_ids = list(range(N_CORES))
    in_maps = []
    for c in core_ids:
        m = {"v1": v1[c * B_CORE:(c + 1) * B_CORE],
             "v2": v2[c * B_CORE:(c + 1) * B_CORE]}
        m.update(weights)
        in_maps.append(m)

    res = run_bass_kernel_spmd(nc, in_maps, core_ids, **run_kwargs)
    hub = np.concatenate([r["hub"] for r in res.results], axis=0)
    out = np.concatenate([r["out"] for r in res.results], axis=0)
    kernel.last_results = res
    return (hub, out)


# revision 6
# speedup vs baseline: 1.1349x; 1.1349x over previous
"""Trainium2 Bass kernel for nn_AxonalConnections.

Computes, for full inputs v1, v2 of shape [32, 1024, 1024] and four
[512, 512] weight maps:
    hub = v1[:, ::2, ::2] * w_v1_hub + v2[:, ::2, ::2] * w_v2_hub
    out = v1[:, ::2, ::2] * w_v1_out + v2[:, ::2, ::2] * w_v2_out

Sharding (8 cores): hybrid 2-way batch x 4-way target-row-block.
Core c = (bg, rg) with bg = c // 4, rg = c % 4 handles images
[16*bg, 16*bg+16) and target rows [128*rg, 128*rg+128). Each core
receives only its source-row slab (rows [256*rg, 256*rg+256)) and its
128-row weight slice, so replicated-weight traffic is 1 MiB/core
instead of 4 MiB.

Per-core kernel (memory-bound design):
  - Only even source rows are read from HBM (stride-2 row DMA, 4 KiB
    contiguous chunks).
  - The even-column gather is folded into the DVE multiply as a
    stride-2 free-dim access pattern (no separate gather pass).
  - 16 images are processed in 4 groups of 4; tiles pack the group
    along the free dim so each DVE op runs at FD=2048 and each input
    DMA is 2 MiB.
  - Outputs are written in a kernel-private layout [128, ig, img, col]
    (8 KiB contiguous DMA chunks); the host reassembles.
"""

import sys

if "/opt/trn_rl_repo" not in sys.path:
    sys.path.insert(0, "/opt/trn_rl_repo")

import numpy as np

N_CORES = 8
B_FULL = 32
SH = SW = 1024
TH = TW = 512
BG = 2            # batch groups
RG = 4            # row groups
B_CORE = B_FULL // BG   # 16 images per core
P = TH // RG            # 128 partitions = target rows per core
IG_B = 4                # images per inner group
N_IG = B_CORE // IG_B   # 4 inner groups

_W_NAMES = ("w_v1_hub", "w_v2_hub", "w_v1_out", "w_v2_out")

_nc_cache = {}


def build_nc(b=B_CORE, ig_b=IG_B, p=P, sw=SW, tw=TW):
    """Build the per-core Bass program. Parameterized so a miniature
    version can be validated in CoreSim.

    Per-core inputs:  v1, v2: [b, 2*p, sw] (source-row slab)
                      w_*: [p, tw]
    Per-core outputs: hub, out: [p, n_ig, ig_b, tw]
                      (target row r = partition, image = ig*ig_b+i)
    """
    from concourse import bacc, mybir
    from concourse.tile import TileContext

    n_ig = b // ig_b
    f32 = mybir.dt.float32
    nc = bacc.Bacc("TRN2", target_bir_lowering=False, debug=False,
                   num_devices=N_CORES)

    v1 = nc.declare_dram_parameter("v1", [b, 2 * p, sw], f32, isOutput=False)
    v2 = nc.declare_dram_parameter("v2", [b, 2 * p, sw], f32, isOutput=False)
    ws = {
        name: nc.declare_dram_parameter(name, [p, tw], f32, isOutput=False)
        for name in _W_NAMES
    }
    hub = nc.declare_dram_parameter("hub", [p, n_ig, ig_b, tw], f32,
                                    isOutput=True)
    out = nc.declare_dram_parameter("out", [p, n_ig, ig_b, tw], f32,
                                    isOutput=True)

    with TileContext(nc) as tc:
        with tc.tile_pool(name="wpool", bufs=1) as wpool, \
             tc.tile_pool(name="inpool", bufs=2) as inpool, \
             tc.tile_pool(name="opool", bufs=2) as opool, \
             tc.tile_pool(name="tpool", bufs=2) as tpool:
            wt = {}
            for name in _W_NAMES:
                t = wpool.tile([p, tw], f32, tag=name)
                nc.sync.dma_start(out=t, in_=ws[name][:, :])
                wt[name] = t

            for ig in range(n_ig):
                i0 = ig * ig_b
                tv1 = inpool.tile([p, ig_b, sw], f32, tag="tv1")
                tv2 = inpool.tile([p, ig_b, sw], f32, tag="tv2")
                nc.sync.dma_start(
                    out=tv1,
                    in_=v1[i0:i0 + ig_b, 0:2 * p:2, :].transpose([1, 0, 2]))
                nc.sync.dma_start(
                    out=tv2,
                    in_=v2[i0:i0 + ig_b, 0:2 * p:2, :].transpose([1, 0, 2]))
                v1e = tv1[:, :, 0:sw:2]  # [p, ig_b, tw] stride-2 col gather
                v2e = tv2[:, :, 0:sw:2]

                for dram_dst, w1n, w2n, otag in (
                        (hub, "w_v1_hub", "w_v2_hub", "thub"),
                        (out, "w_v1_out", "w_v2_out", "tout")):
                    to = opool.tile([p, ig_b, tw], f32, tag=otag)
                    tt = tpool.tile([p, ig_b, tw], f32, tag="tmp")
                    w1 = wt[w1n].unsqueeze(1).broadcast_to([p, ig_b, tw])
                    w2 = wt[w2n].unsqueeze(1).broadcast_to([p, ig_b, tw])
                    nc.vector.tensor_mul(out=to, in0=v1e, in1=w1)
                    nc.vector.tensor_mul(out=tt, in0=v2e, in1=w2)
                    nc.vector.tensor_add(out=to, in0=to, in1=tt)
                    nc.sync.dma_start(out=dram_dst[:, ig, :, :], in_=to)

    nc.compile()
    return nc


def _get_nc():
    if "full" not in _nc_cache:
        _nc_cache["full"] = build_nc()
    return _nc_cache["full"]


def kernel(v1, v2, w_v1_hub, w_v2_hub, w_v1_out, w_v2_out, **run_kwargs):
    """Full-input entry point: shards over (batch-group, row-group),
    runs on 8 cores, gathers full outputs. Returns (hub, out)."""
    from concourse.bass_utils import run_bass_kernel_spmd

    nc = _get_nc()
    v1 = np.asarray(v1, dtype=np.float32)
    v2 = np.asarray(v2, dtype=np.float32)
    wfull = {
        "w_v1_hub": np.asarray(w_v1_hub, np.float32),
        "w_v2_hub": np.asarray(w_v2_hub, np.float32),
        "w_v1_out": np.asarray(w_v1_out, np.float32),
        "w_v2_out": np.asarray(w_v2_out, np.float32),
    }

    core_ids = list(range(N_CORES))
    in_maps = []
    for c in core_ids:
        bg, rg = divmod(c, RG)
        bsl = slice(bg * B_CORE, (bg + 1) * B_CORE)
        rsl = slice(rg * 2 * P, (rg + 1) * 2 * P)
        m = {"v1": np.ascontiguousarray(v1[bsl, rsl, :]),
             "v2": np.ascontiguousarray(v2[bsl, rsl, :])}
        for name, w in wfull.items():
            m[name] = np.ascontiguousarray(w[rg * P:(rg + 1) * P, :])
        in_maps.append(m)

    res = run_bass_kernel_spmd(nc, in_maps, core_ids, **run_kwargs)

    hub = np.empty((B_FULL, TH, TW), np.float32)
    out = np.empty((B_FULL, TH, TW), np.float32)
    for c in core_ids:
        bg, rg = divmod(c, RG)
        for name, full in (("hub", hub), ("out", out)):
            buf = res.results[c][name]  # [P, N_IG, IG_B, TW]
            img_major = buf.transpose(1, 2, 0, 3).reshape(B_CORE, P, TW)
            full[bg * B_CORE:(bg + 1) * B_CORE,
                 rg * P:(rg + 1) * P, :] = img_major
    kernel.last_results = res
    return (hub, out)
